# revision 1
# baseline (speedup 1.0000x reference)
"""Multi-head attention (B=2, S=4096, D=768, H=12) on 8 Trainium2 cores.

Sharding: core c handles batch c//4 and heads 3*(c%4)..3*(c%4)+2.
Each core computes its 3 heads end-to-end (QKV projection, causal
attention, its partial of the output projection); the host sums the 4
per-batch partials and adds the output bias.

Device algorithm per core (S=4096, DK=64, 3 heads), bf16 matmuls with
fp32 PSUM accumulation:
  - Q^T,K^T [e,s] via matmul chains (contract d=768), heads 0,1 packed on
    partitions 0-63/64-127 of slot 0, head 2 on partitions 0-63 of slot 1.
  - V [s,e] per head with a ones column appended (65 wide).
  - scores^T tiles [128k, 512q] per (key-block, query-group).
    Only causally-live key blocks are computed; diagonal blocks get exp()
    then a 0/1 mask multiply (exact zeros).
  - ctx^T + softmax denominator in one accumulating matmul:
    lhsT=[V|1] (65 cols) -> psum rows 0-63 ctx^T, row 64 = sum(exp).
  - denominator reciprocal broadcast across partitions via a K=1 matmul
    with a ones row, then one DVE multiply normalizes into bf16 ctx^T.
  - output projection from ctx^T (bf16) against wo^T slices.
"""

import sys

sys.path.insert(0, "/opt/trn_rl_repo")

import ml_dtypes
import numpy as np

import concourse.bass as bass
import concourse.mybir as mybir
import concourse.tile as tile
from concourse.bass_utils import run_bass_kernel_spmd

B, S, D, H = 2, 4096, 768, 12
DK = D // H          # 64
NCORES = 8
HPC = 3              # heads per core
E = HPC * DK         # 192 = per-core projection width
P = 128
DC = D // P          # 6 contraction chunks of 128
SG = S // 512        # 8 query groups of 512
SC = S // P          # 32 token chunks of 128
F32 = mybir.dt.float32
F32R = mybir.dt.float32r
BF16 = mybir.dt.bfloat16
EXP = mybir.ActivationFunctionType.Exp
IDENT = mybir.ActivationFunctionType.Identity
BF = ml_dtypes.bfloat16


def _split_multi_waits(nc):
    """This walrus build encodes exactly one sync wait per TPB instruction
    and refuses to split multi-wait instructions itself. Rewrite each block
    so extra waits land on same-engine NOPs directly before the owner."""
    k = 0
    for f in nc.m.functions:
        for blk in f.blocks:
            out = []
            changed = False
            for inst in blk.instructions:
                si = inst.sync_info
                if si is not None and len(si.on_wait) > 1:
                    changed = True
                    waits = list(si.on_wait)
                    for w in waits[:-1]:
                        nop = mybir.InstNoOp(name=f"splitw-{k}", ins=[], outs=[])
                        k += 1
                        nop.engine = inst.engine
                        nop.sync_info = mybir.SyncInfo(on_wait=[w], on_update=[])
                        out.append(nop)
                    inst.sync_info = mybir.SyncInfo(
                        on_wait=[waits[-1]], on_update=list(si.on_update)
                    )
                out.append(inst)
            if changed:
                blk.instructions = out


def _r(ap):
    return ap.bitcast(F32R)


def _build_program(repeat=1, parts="all"):
    nc = bass.Bass("TRN2", target_bir_lowering=False, debug=False)

    qT = nc.declare_dram_parameter("qT", [D, S], BF16, isOutput=False)
    kT = nc.declare_dram_parameter("kT", [D, S], BF16, isOutput=False)
    vT = nc.declare_dram_parameter("vT", [D, S], BF16, isOutput=False)
    wqT = nc.declare_dram_parameter("wqT", [D, E], BF16, isOutput=False)
    wkT = nc.declare_dram_parameter("wkT", [D, E], BF16, isOutput=False)
    wvT = nc.declare_dram_parameter("wvT", [D, 256], BF16, isOutput=False)
    woT = nc.declare_dram_parameter("woT", [E, D], BF16, isOutput=False)
    bq = nc.declare_dram_parameter("bq", [P, 2], F32, isOutput=False)  # (bq)/8 packed
    bk = nc.declare_dram_parameter("bk", [P, 2], F32, isOutput=False)
    bv = nc.declare_dram_parameter("bv", [P, 256], F32, isOutput=False)
    maskc = nc.declare_dram_parameter("maskc", [P, 4 * 512], BF16, isOutput=False)
    out_p = nc.declare_dram_parameter("out_p", [S, D], BF16, isOutput=True)

    qT_r = qT[:].rearrange("(dc p) s -> p dc s", p=P)
    kT_r = kT[:].rearrange("(dc p) s -> p dc s", p=P)
    vT_r = vT[:].rearrange("(dc p) s -> p dc s", p=P)

    rp = repeat if parts in ("all", "proj") else 1
    ra = repeat if parts in ("all", "attn") else 1

    with tile.TileContext(nc) as tc:
        import contextlib

        with contextlib.ExitStack() as ctx:
            const = ctx.enter_context(tc.tile_pool(name="const", bufs=1))
            persist = ctx.enter_context(tc.tile_pool(name="persist", bufs=1))

            # ---- constants ----
            wq_sb = const.tile([P, DC, E], BF16)
            nc.sync.dma_start(wq_sb[:], wqT[:].rearrange("(dc p) e -> p dc e", p=P))
            wk_sb = const.tile([P, DC, E], BF16)
            nc.sync.dma_start(wk_sb[:], wkT[:].rearrange("(dc p) e -> p dc e", p=P))
            wv_sb = const.tile([P, DC, 256], BF16)
            nc.sync.dma_start(wv_sb[:], wvT[:].rearrange("(dc p) e -> p dc e", p=P))
            wo_sb = const.tile([64, HPC, D], BF16)
            nc.sync.dma_start(wo_sb[:], woT[:].rearrange("(h p) o -> p h o", p=64))
            bq_sb = const.tile([P, 2], F32)
            nc.sync.dma_start(bq_sb[:], bq[:])
            bk_sb = const.tile([P, 2], F32)
            nc.sync.dma_start(bk_sb[:], bk[:])
            bv_sb = const.tile([P, 256], F32)
            nc.sync.dma_start(bv_sb[:], bv[:])
            mask_sb = const.tile([P, 4 * 512], BF16)
            nc.sync.dma_start(mask_sb[:], maskc[:])
            onesf = const.tile([P, 64], F32)
            nc.any.memset(onesf[:], 1.0)
            ones_sb = const.tile([P, 64], F32R)
            nc.vector.tensor_copy(ones_sb[:], onesf[:])
            onesb = const.tile([P, 64], BF16)
            nc.vector.tensor_copy(onesb[:], onesf[:])

            # ---- persistent activations ----
            # slot 0: heads 0 (parts 0-63) & 1 (parts 64-127); slot 1: head 2 low.
            QT_sb = persist.tile([P, 2, S], BF16)
            KT_sb = persist.tile([P, 2, S], BF16)
            V_sb = [
                persist.tile([P, SC, 65], BF16, tag=f"v{h}", name=f"v{h}")
                for h in range(HPC)
            ]
            ctxT_sb = persist.tile([64, HPC, S], BF16)

            for h in range(HPC):
                nc.vector.tensor_copy(V_sb[h][:, :, 64], onesb[:, 0:SC])

            # (slot, base partition, rows) for each head's Q/K storage
            head_loc = [(0, 0, 64), (0, 64, 64), (1, 0, 64)]

            # ---- phase 1: Q/K projections;  phase 2: V projection ----
            for _rp in range(rp):
                with tc.tile_pool(name="pj", bufs=4) as pj, \
                     tc.tile_pool(name="pjp", bufs=4, space="PSUM") as pjp:
                    for (w_sb, b_sb, dst, scale) in (
                        (wq_sb, bq_sb, QT_sb, 0.125),
                        (wk_sb, bk_sb, KT_sb, 1.0),
                    ):
                        src = qT_r if dst is QT_sb else kT_r
                        for sg2 in range(SG // 2):
                            stg = pj.tile([P, DC, 1024], BF16, tag="stage")
                            nc.sync.dma_start(
                                stg[:], src[:, :, sg2 * 1024:(sg2 + 1) * 1024]
                            )
                            for half in range(2):
                                sg = 2 * sg2 + half
                                for ec, em in ((0, P), (1, 64)):
                                    ps = pjp.tile([P, 512], F32, tag="qk")
                                    for dc in range(DC):
                                        nc.tensor.matmul(
                                            ps[:em, :],
                                            w_sb[:, dc, ec * P:ec * P + em],
                                            stg[:, dc, half * 512:(half + 1) * 512],
                                            start=(dc == 0), stop=(dc == DC - 1),
                                        )
                                    nc.scalar.activation(
                                        dst[0:em, ec, sg * 512:(sg + 1) * 512]
                                        if ec == 1
                                        else dst[:, 0, sg * 512:(sg + 1) * 512],
                                        ps[:em, :],
                                        IDENT,
                                        bias=b_sb[:em, ec:ec + 1],
                                        scale=scale,
                                    )

                    for sc4 in range(SC // 4):
                        vstg = pj.tile([P, DC, 512], BF16, tag="vstage")
                        nc.sync.dma_start(
                            vstg[:], vT_r[:, :, sc4 * 512:(sc4 + 1) * 512]
                        )
                        for quart in range(4):
                            sc = 4 * sc4 + quart
                            ps = pjp.tile([P, 256], F32, tag="v")
                            for dc in range(DC):
                                nc.tensor.matmul(
                                    ps[:],
                                    vstg[:, dc, quart * P:(quart + 1) * P],
                                    wv_sb[:, dc, :],
                                    start=(dc == 0), stop=(dc == DC - 1),
                                )
                            for h in range(HPC):
                                nc.vector.tensor_tensor(
                                    V_sb[h][:, sc, 0:64],
                                    ps[:, h * 64:(h + 1) * 64],
                                    bv_sb[:, h * 64:(h + 1) * 64],
                                    mybir.AluOpType.add,
                                )

            # ---- phase 3: attention;  phase 4: output projection ----
            for _ra in range(ra):
                with tc.tile_pool(name="att", bufs=8) as att, \
                     tc.tile_pool(name="nrm", bufs=3) as nrm, \
                     tc.tile_pool(name="stp", bufs=4, space="PSUM") as stp, \
                     tc.tile_pool(name="ctxp", bufs=3, space="PSUM") as ctxp, \
                     tc.tile_pool(name="bcp", bufs=1, space="PSUM") as bcp:
                    for qg in range(SG):
                        nkb = 4 * (qg + 1)
                        ctx_ps = {}
                        for h in range(HPC):
                            ctx_ps[h] = ctxp.tile(
                                [P, 512], F32, tag="ctx", name=f"ctx{h}"
                            )
                        # one [128,512] score tile per key-block: 4 PSUM slots
                        # in flight so no head ever stalls on exp draining.
                        for kb in range(nkb):
                            # all 3 heads' score matmuls back-to-back: heads
                            # 0/1 use PE row groups 0-63/64-127 and execute
                            # concurrently; 4 PSUM slots so none stalls on exp.
                            sts = {}
                            for h in range(HPC):
                                slot, p0, rows = head_loc[h]
                                sts[h] = stp.tile([P, 512], F32, tag="st",
                                                  name=f"st{h}")
                                nc.tensor.matmul(
                                    sts[h][:],
                                    KT_sb[p0:p0 + rows, slot, kb * P:(kb + 1) * P],
                                    QT_sb[p0:p0 + rows, slot,
                                          qg * 512:(qg + 1) * 512],
                                    start=True, stop=True,
                                )
                            ets = {}
                            for h in range(HPC):
                                et = att.tile([P, 512], BF16, tag="et")
                                if kb >= nkb - 4:
                                    # diagonal: exp then 0/1 mask multiply
                                    ete = att.tile([P, 512], BF16, tag="ete")
                                    nc.scalar.activation(ete[:], sts[h][:], EXP)
                                    j = kb - (nkb - 4)
                                    nc.vector.tensor_tensor(
                                        et[:], ete[:],
                                        mask_sb[:, j * 512:(j + 1) * 512],
                                        mybir.AluOpType.mult,
                                    )
                                else:
                                    nc.scalar.activation(et[:], sts[h][:], EXP)
                                ets[h] = et
                            for h in range(HPC):
                                nc.tensor.matmul(
                                    ctx_ps[h][0:65, :],
                                    V_sb[h][:, kb, :],
                                    ets[h][:],
                                    start=(kb == 0), stop=(kb == nkb - 1),
                                )
                        for h in range(HPC):
                            rc = nrm.tile([P, 512], F32R, tag="rc")
                            with nc.allow_low_precision(
                                reason="softmax denominator reciprocal; f32r "
                                "rounding is benign here"
                            ):
                                nc.vector.reciprocal(
                                    rc[64:65, :], ctx_ps[h][64:65, :]
                                )
                            bc = bcp.tile([64, 512], F32, tag="bc")
                            nc.tensor.matmul(
                                bc[:], _r(ones_sb[64:65, :]), rc[64:65, :],
                                start=True, stop=True,
                            )
                            rcb = nrm.tile([64, 512], F32, tag="rcb")
                            nc.vector.tensor_copy(rcb[:], bc[:])
                            nc.vector.tensor_tensor(
                                ctxT_sb[0:64, h, qg * 512:(qg + 1) * 512],
                                ctx_ps[h][0:64, :],
                                rcb[:],
                                mybir.AluOpType.mult,
                            )

                with tc.tile_pool(name="ob", bufs=3) as ob, \
                     tc.tile_pool(name="op", bufs=2, space="PSUM") as op:
                    for sc in range(SC):
                        osb = ob.tile([P, D], BF16, tag="osb")
                        for og, o0, ow in ((0, 0, 512), (1, 512, 256)):
                            ps = op.tile([P, 512], F32, tag=f"og{og}")
                            for h in range(HPC):
                                nc.tensor.matmul(
                                    ps[:, :ow],
                                    ctxT_sb[0:64, h, sc * P:(sc + 1) * P],
                                    wo_sb[:, h, o0:o0 + ow],
                                    start=(h == 0), stop=(h == HPC - 1),
                                )
                            if og == 0:
                                nc.vector.tensor_copy(osb[:, o0:o0 + ow], ps[:, :ow])
                            else:
                                nc.scalar.copy(osb[:, o0:o0 + ow], ps[:, :ow])
                        nc.sync.dma_start(out_p[sc * P:(sc + 1) * P, :], osb[:])

    _split_multi_waits(nc)
    return nc


_CACHED_NC = None


def _get_nc():
    global _CACHED_NC
    if _CACHED_NC is None:
        _CACHED_NC = _build_program()
    return _CACHED_NC


def _numpy_reference(q, k, v, wq, bq, wk, bk, wv, bv, wo, bo, mask):
    """Fallback for masks the fast path does not handle (non-causal)."""
    out = np.empty((B, S, D), np.float32)
    scale = 1.0 / np.sqrt(DK)
    for b in range(B):
        Q = (q[b] @ wq.T + bq).reshape(S, H, DK).transpose(1, 0, 2)
        K = (k[b] @ wk.T + bk).reshape(S, H, DK).transpose(1, 0, 2)
        V = (v[b] @ wv.T + bv).reshape(S, H, DK).transpose(1, 0, 2)
        ctx = np.empty((H, S, DK), np.float32)
        for h in range(H):
            s = (Q[h] @ K[h].T) * scale
            s = np.where(mask, s, -1e9)
            s -= s.max(axis=-1, keepdims=True)
            e = np.exp(s)
            p = e / e.sum(axis=-1, keepdims=True)
            ctx[h] = p @ V[h]
        out[b] = ctx.transpose(1, 0, 2).reshape(S, D) @ wo.T + bo
    return out


def _prepare_in_maps(q, k, v, wq, bq, wk, bk, wv, bv, wo):
    # causal 0/1 diagonal-block masks: maskc[k, j*512+q] = (128j + k) <= q
    kk = np.arange(P)[:, None]
    qq = np.arange(512)[None, :]
    maskc = np.zeros((P, 4, 512), np.float32)
    for j in range(4):
        maskc[:, j, :] = (P * j + kk) <= qq
    maskc = np.ascontiguousarray(maskc.reshape(P, 4 * 512)).astype(BF)

    wqT = np.ascontiguousarray(wq.T).astype(BF)  # [d_in, e_out]
    wkT = np.ascontiguousarray(wk.T).astype(BF)
    wvT = np.ascontiguousarray(wv.T).astype(BF)
    woT = np.ascontiguousarray(wo.T)             # [e_in, d_out]

    qTb = [np.ascontiguousarray(q[b].T).astype(BF) for b in range(B)]
    kTb = [np.ascontiguousarray(k[b].T).astype(BF) for b in range(B)]
    vTb = [np.ascontiguousarray(v[b].T).astype(BF) for b in range(B)]

    def pack_bias(bvec, scale):
        t = np.zeros((P, 2), np.float32)
        t[:, 0] = bvec[:P] * scale
        t[:64, 1] = bvec[P:E] * scale
        return t

    in_maps = []
    for c in range(NCORES):
        b = c // 4
        e0 = 3 * (c % 4) * DK
        wvp = np.zeros((D, 256), BF)
        wvp[:, :E] = wvT[:, e0:e0 + E]
        bvp = np.zeros((P, 256), np.float32)
        bvp[:, :E] = bv[e0:e0 + E][None, :]
        in_maps.append({
            "qT": qTb[b],
            "kT": kTb[b],
            "vT": vTb[b],
            "wqT": np.ascontiguousarray(wqT[:, e0:e0 + E]),
            "wkT": np.ascontiguousarray(wkT[:, e0:e0 + E]),
            "wvT": wvp,
            "woT": np.ascontiguousarray(woT[e0:e0 + E, :]).astype(BF),
            "bq": pack_bias(bq[e0:e0 + E], 0.125),
            "bk": pack_bias(bk[e0:e0 + E], 1.0),
            "bv": bvp,
            "maskc": maskc,
        })
    return in_maps


def kernel(q, k, v, wq, bq, wk, bk, wv, bv, wo, bo, mask, **_unused):
    q = np.asarray(q, np.float32)
    k = np.asarray(k, np.float32)
    v = np.asarray(v, np.float32)
    wq = np.asarray(wq, np.float32)
    wk = np.asarray(wk, np.float32)
    wv = np.asarray(wv, np.float32)
    wo = np.asarray(wo, np.float32)
    bq = np.asarray(bq, np.float32)
    bk = np.asarray(bk, np.float32)
    bv = np.asarray(bv, np.float32)
    bo = np.asarray(bo, np.float32)
    mask = np.asarray(mask)

    tril = np.tril(np.ones((S, S), bool))
    if mask.shape != (S, S) or not np.array_equal(mask.astype(bool), tril):
        return _numpy_reference(q, k, v, wq, bq, wk, bk, wv, bv, wo, bo, mask)

    in_maps = _prepare_in_maps(q, k, v, wq, bq, wk, bk, wv, bv, wo)
    nc = _get_nc()
    res = run_bass_kernel_spmd(nc, in_maps, core_ids=list(range(NCORES)))

    out = np.empty((B, S, D), np.float32)
    for b in range(B):
        acc = res.results[4 * b]["out_p"].astype(np.float32)
        for c in range(4 * b + 1, 4 * b + 4):
            acc = acc + res.results[c]["out_p"].astype(np.float32)
        out[b] = acc + bo[None, :]
    return out



# revision 6
# speedup vs baseline: 7.0422x; 7.0422x over previous
"""Multi-head attention (B=2, S=4096, D=768, H=12) on 8 Trainium2 cores.

Sharding: core c handles batch c//4 and heads 3*(c%4)..3*(c%4)+2.
Each core computes its 3 heads end-to-end (QKV projection, causal
attention, its partial of the output projection); the host sums the 4
per-batch partials and adds the output bias.

Device algorithm per core (S=4096, DK=64, 3 heads), bf16 matmuls with
fp32 PSUM accumulation:
  - Q^T,K^T [e,s] via matmul chains (contract d=768), heads 0,1 packed on
    partitions 0-63/64-127 of slot 0, head 2 on partitions 0-63 of slot 1.
  - V [s,e] per head with a ones column appended (65 wide).
  - scores^T tiles [128k, 512q] per (key-block, query-group).
    Only causally-live key blocks are computed; diagonal blocks get exp()
    then a 0/1 mask multiply (exact zeros).
  - ctx^T + softmax denominator in one accumulating matmul:
    lhsT=[V|1] (65 cols) -> psum rows 0-63 ctx^T, row 64 = sum(exp).
  - denominator reciprocal broadcast across partitions via a K=1 matmul
    with a ones row, then one DVE multiply normalizes into bf16 ctx^T.
  - output projection from ctx^T (bf16) against wo^T slices.
"""

import sys

sys.path.insert(0, "/opt/trn_rl_repo")

import ml_dtypes
import numpy as np

import concourse.bass as bass
import concourse.mybir as mybir
import concourse.tile as tile
from concourse.bass_utils import run_bass_kernel_spmd

B, S, D, H = 2, 4096, 768, 12
DK = D // H          # 64
NCORES = 8
HPC = 3              # heads per core
E = HPC * DK         # 192 = per-core projection width
P = 128
DC = D // P          # 6 contraction chunks of 128
SG = S // 512        # 8 query groups of 512
SC = S // P          # 32 token chunks of 128
F32 = mybir.dt.float32
F32R = mybir.dt.float32r
BF16 = mybir.dt.bfloat16
EXP = mybir.ActivationFunctionType.Exp
IDENT = mybir.ActivationFunctionType.Identity
BF = ml_dtypes.bfloat16


def _split_multi_waits(nc):
    """This walrus build encodes exactly one sync wait per TPB instruction
    and refuses to split multi-wait instructions itself. Rewrite each block
    so extra waits land on same-engine NOPs directly before the owner."""
    k = 0
    for f in nc.m.functions:
        for blk in f.blocks:
            out = []
            changed = False
            for inst in blk.instructions:
                si = inst.sync_info
                if si is not None and len(si.on_wait) > 1:
                    changed = True
                    waits = list(si.on_wait)
                    for w in waits[:-1]:
                        nop = mybir.InstNoOp(name=f"splitw-{k}", ins=[], outs=[])
                        k += 1
                        nop.engine = inst.engine
                        nop.sync_info = mybir.SyncInfo(on_wait=[w], on_update=[])
                        out.append(nop)
                    inst.sync_info = mybir.SyncInfo(
                        on_wait=[waits[-1]], on_update=list(si.on_update)
                    )
                out.append(inst)
            if changed:
                blk.instructions = out


def _r(ap):
    return ap.bitcast(F32R)


def _build_program(repeat=1, parts="all"):
    nc = bass.Bass("TRN2", target_bir_lowering=False, debug=False)

    qT = nc.declare_dram_parameter("qT", [D, S], BF16, isOutput=False)
    kT = nc.declare_dram_parameter("kT", [D, S], BF16, isOutput=False)
    vT = nc.declare_dram_parameter("vT", [D, S], BF16, isOutput=False)
    wqT = nc.declare_dram_parameter("wqT", [D, E], BF16, isOutput=False)
    wkT = nc.declare_dram_parameter("wkT", [D, E], BF16, isOutput=False)
    wvT = nc.declare_dram_parameter("wvT", [D, 256], BF16, isOutput=False)
    woT = nc.declare_dram_parameter("woT", [E, D], BF16, isOutput=False)
    bq = nc.declare_dram_parameter("bq", [P, 2], F32, isOutput=False)  # (bq)/8 packed
    bk = nc.declare_dram_parameter("bk", [P, 2], F32, isOutput=False)
    bv = nc.declare_dram_parameter("bv", [P, 256], F32, isOutput=False)
    maskc = nc.declare_dram_parameter("maskc", [P, 4 * 512], BF16, isOutput=False)
    out_p = nc.declare_dram_parameter("out_p", [S, D], BF16, isOutput=True)

    qT_r = qT[:].rearrange("(dc p) s -> p dc s", p=P)
    kT_r = kT[:].rearrange("(dc p) s -> p dc s", p=P)
    vT_r = vT[:].rearrange("(dc p) s -> p dc s", p=P)

    rp = repeat if parts in ("all", "proj") else 1
    ra = repeat if parts in ("all", "attn") else 1

    with tile.TileContext(nc) as tc:
        import contextlib

        with contextlib.ExitStack() as ctx:
            const = ctx.enter_context(tc.tile_pool(name="const", bufs=1))
            persist = ctx.enter_context(tc.tile_pool(name="persist", bufs=1))

            # ---- constants ----
            wq_sb = const.tile([P, DC, E], BF16)
            nc.sync.dma_start(wq_sb[:], wqT[:].rearrange("(dc p) e -> p dc e", p=P))
            wk_sb = const.tile([P, DC, E], BF16)
            nc.sync.dma_start(wk_sb[:], wkT[:].rearrange("(dc p) e -> p dc e", p=P))
            wv_sb = const.tile([P, DC, 256], BF16)
            nc.sync.dma_start(wv_sb[:], wvT[:].rearrange("(dc p) e -> p dc e", p=P))
            wo_sb = const.tile([64, HPC, D], BF16)
            nc.sync.dma_start(wo_sb[:], woT[:].rearrange("(h p) o -> p h o", p=64))
            bq_sb = const.tile([P, 2], F32)
            nc.sync.dma_start(bq_sb[:], bq[:])
            bk_sb = const.tile([P, 2], F32)
            nc.sync.dma_start(bk_sb[:], bk[:])
            bv_sb = const.tile([P, 256], F32)
            nc.sync.dma_start(bv_sb[:], bv[:])
            mask_sb = const.tile([P, 4 * 512], BF16)
            nc.sync.dma_start(mask_sb[:], maskc[:])
            onesf = const.tile([P, 64], F32)
            nc.any.memset(onesf[:], 1.0)
            ones_sb = const.tile([P, 64], F32R)
            nc.vector.tensor_copy(ones_sb[:], onesf[:])
            onesb = const.tile([P, 64], BF16)
            nc.vector.tensor_copy(onesb[:], onesf[:])

            # ---- persistent activations ----
            # slot 0: heads 0 (parts 0-63) & 1 (parts 64-127); slot 1: head 2 low.
            QT_sb = persist.tile([P, 2, S], BF16)
            KT_sb = persist.tile([P, 2, S], BF16)
            V_sb = [
                persist.tile([P, SC, 65], BF16, tag=f"v{h}", name=f"v{h}")
                for h in range(HPC)
            ]
            ctxT_sb = persist.tile([64, HPC, S], BF16)

            for h in range(HPC):
                nc.vector.tensor_copy(V_sb[h][:, :, 64], onesb[:, 0:SC])

            # (slot, base partition, rows) for each head's Q/K storage
            head_loc = [(0, 0, 64), (0, 64, 64), (1, 0, 64)]

            # ---- phase 1: Q/K projections;  phase 2: V projection ----
            for _rp in range(rp):
                with tc.tile_pool(name="pj", bufs=4) as pj, \
                     tc.tile_pool(name="pjp", bufs=4, space="PSUM") as pjp:
                    for (w_sb, b_sb, dst, scale) in (
                        (wq_sb, bq_sb, QT_sb, 0.125),
                        (wk_sb, bk_sb, KT_sb, 1.0),
                    ):
                        src = qT_r if dst is QT_sb else kT_r
                        for sg2 in range(SG // 2):
                            stg = pj.tile([P, DC, 1024], BF16, tag="stage")
                            if dst is QT_sb and sg2 == 0:
                                # split the very first stage so PE starts on
                                # the first half while the rest still streams
                                nc.sync.dma_start(
                                    stg[:, :, 0:512], src[:, :, 0:512]
                                )
                                nc.sync.dma_start(
                                    stg[:, :, 512:1024], src[:, :, 512:1024]
                                )
                            else:
                                nc.sync.dma_start(
                                    stg[:],
                                    src[:, :, sg2 * 1024:(sg2 + 1) * 1024],
                                )
                            for half in range(2):
                                sg = 2 * sg2 + half
                                for ec, em in ((0, P), (1, 64)):
                                    ps = pjp.tile([P, 512], F32, tag="qk")
                                    for dc in range(DC):
                                        nc.tensor.matmul(
                                            ps[:em, :],
                                            w_sb[:, dc, ec * P:ec * P + em],
                                            stg[:, dc, half * 512:(half + 1) * 512],
                                            start=(dc == 0), stop=(dc == DC - 1),
                                        )
                                    nc.scalar.activation(
                                        dst[0:em, ec, sg * 512:(sg + 1) * 512]
                                        if ec == 1
                                        else dst[:, 0, sg * 512:(sg + 1) * 512],
                                        ps[:em, :],
                                        IDENT,
                                        bias=b_sb[:em, ec:ec + 1],
                                        scale=scale,
                                    )
                            # duplicate head 2 onto partitions 64-127 of slot
                            # 1 so its score matmuls can pair across PE row
                            # groups (one SBUF->SBUF DMA per 1024 columns)
                            win = slice(sg2 * 1024, (sg2 + 1) * 1024)
                            nc.sync.dma_start(
                                dst[64:128, 1, win], dst[0:64, 1, win]
                            )

                    for sc4 in range(SC // 4):
                        vstg = pj.tile([P, DC, 512], BF16, tag="vstage")
                        nc.sync.dma_start(
                            vstg[:], vT_r[:, :, sc4 * 512:(sc4 + 1) * 512]
                        )
                        for quart in range(4):
                            sc = 4 * sc4 + quart
                            ps = pjp.tile([P, 256], F32, tag="v")
                            for dc in range(DC):
                                nc.tensor.matmul(
                                    ps[:],
                                    vstg[:, dc, quart * P:(quart + 1) * P],
                                    wv_sb[:, dc, :],
                                    start=(dc == 0), stop=(dc == DC - 1),
                                )
                            for h in range(HPC):
                                nc.vector.tensor_tensor(
                                    V_sb[h][:, sc, 0:64],
                                    ps[:, h * 64:(h + 1) * 64],
                                    bv_sb[:, h * 64:(h + 1) * 64],
                                    mybir.AluOpType.add,
                                )

            # ---- phase 3: attention + inline output projection ----
            # Key blocks processed in PAIRS: one [128, 2, 512] fp32 PSUM tile
            # (2 banks) per (head, pair); one exp covers both blocks (N=1024),
            # halving ScalarE instruction overhead. Output projection runs
            # inline after each query group, its PSUM tiles reusing the ctx
            # slots (same tag), so its PE/DVE work hides under the ACT-bound
            # attention stream of the next group.
            for _ra in range(ra):
                with tc.tile_pool(name="att", bufs=4) as att, \
                     tc.tile_pool(name="nrm", bufs=3) as nrm, \
                     tc.tile_pool(name="ob", bufs=3) as ob, \
                     tc.tile_pool(name="stp", bufs=2, space="PSUM") as stp, \
                     tc.tile_pool(name="ctxp", bufs=3, space="PSUM") as ctxp, \
                     tc.tile_pool(name="bcp", bufs=1, space="PSUM") as bcp:
                    def emit_scores_exps(qg, pr):
                        """Score matmuls + batched exp for one kb pair.
                        Heads 0/1 pair across PE row groups; head 2 pairs its
                        two key blocks across row groups via the duplicated
                        slot-1 upper half."""
                        qwin = QT_sb[:, :, qg * 512:(qg + 1) * 512]
                        kbs = (2 * pr, 2 * pr + 1)
                        sts = {}
                        for h in (0, 1):
                            sts[h] = stp.tile([P, 2, 512], F32, tag="st",
                                              name=f"st{h}")
                        for j, kb in enumerate(kbs):
                            for h in (0, 1):
                                slot, p0, rows = head_loc[h]
                                nc.tensor.matmul(
                                    sts[h][:, j, :],
                                    KT_sb[p0:p0 + rows, slot,
                                          kb * P:(kb + 1) * P],
                                    qwin[p0:p0 + rows, slot, :],
                                    start=True, stop=True,
                                )
                        ets = {}
                        for h in (0, 1):
                            ets[h] = att.tile([P, 2, 512], BF16, tag="et",
                                              name=f"et{h}")
                            nc.scalar.activation(ets[h][:], sts[h][:], EXP)
                        sts[2] = stp.tile([P, 2, 512], F32, tag="st",
                                          name="st2")
                        for j, kb in enumerate(kbs):
                            p0 = 64 * j
                            nc.tensor.matmul(
                                sts[2][:, j, :],
                                KT_sb[p0:p0 + 64, 1, kb * P:(kb + 1) * P],
                                qwin[p0:p0 + 64, 1, :],
                                start=True, stop=True,
                            )
                        ets[2] = att.tile([P, 2, 512], BF16, tag="et",
                                          name="et2")
                        nc.scalar.activation(ets[2][:], sts[2][:], EXP)
                        return ets

                    def emit_mask_ctx(qg, pr, ets, ctx_ps):
                        """Diagonal 0/1 masking + ctx accumulation matmuls."""
                        nkb = 4 * (qg + 1)
                        kbs = (2 * pr, 2 * pr + 1)
                        for j, kb in enumerate(kbs):
                            if kb >= nkb - 4:
                                dj = kb - (nkb - 4)
                                for h in range(HPC):
                                    em = att.tile([P, 512], BF16,
                                                  tag="etm", name="em")
                                    nc.vector.tensor_tensor(
                                        em[:], ets[h][:, j, :],
                                        mask_sb[:, dj * 512:(dj + 1) * 512],
                                        mybir.AluOpType.mult,
                                    )
                                    ets[(h, j)] = em
                        for h in range(HPC):
                            for j, kb in enumerate(kbs):
                                src = ets.get((h, j), None)
                                esl = (src[:] if src is not None
                                       else ets[h][:, j, :])
                                nc.tensor.matmul(
                                    ctx_ps[h][0:65, :],
                                    V_sb[h][:, kb, :],
                                    esl,
                                    start=(kb == 0), stop=(kb == nkb - 1),
                                )

                    def emit_outproj(src_qg):
                        """Output projection for one query group; PSUM tiles
                        share the ctx slots (same tag)."""
                        for sc in range(4 * src_qg, 4 * src_qg + 4):
                            osb = ob.tile([P, D], BF16, tag="osb")
                            for og, o0, ow in ((0, 0, 512), (1, 512, 256)):
                                ps = ctxp.tile([P, 512], F32, tag="ctx",
                                               name=f"og{og}")
                                for h in range(HPC):
                                    nc.tensor.matmul(
                                        ps[:, :ow],
                                        ctxT_sb[0:64, h, sc * P:(sc + 1) * P],
                                        wo_sb[:, h, o0:o0 + ow],
                                        start=(h == 0), stop=(h == HPC - 1),
                                    )
                                nc.vector.tensor_copy(
                                    osb[:, o0:o0 + ow], ps[:, :ow]
                                )
                            nc.sync.dma_start(
                                out_p[sc * P:(sc + 1) * P, :], osb[:]
                            )

                    # out-projection of group qg is emitted after the first
                    # score/exp pair of group qg+1: its matmuls fill the PE
                    # idle window while qg+1's first exp drains, and its ctx-
                    # slot allocations rotate in cleanly after qg's ctx tiles.
                    pending = None
                    for qg in range(SG):
                        nkb = 4 * (qg + 1)
                        ets0 = emit_scores_exps(qg, 0)
                        if pending is not None:
                            emit_outproj(pending)
                            pending = None
                        ctx_ps = {}
                        for h in range(HPC):
                            ctx_ps[h] = ctxp.tile(
                                [P, 512], F32, tag="ctx", name=f"ctx{h}"
                            )
                        emit_mask_ctx(qg, 0, ets0, ctx_ps)
                        for pr in range(1, nkb // 2):
                            ets = emit_scores_exps(qg, pr)
                            emit_mask_ctx(qg, pr, ets, ctx_ps)
                        for h in range(HPC):
                            rc = nrm.tile([P, 512], F32R, tag="rc")
                            with nc.allow_low_precision(
                                reason="softmax denominator reciprocal; f32r "
                                "rounding is benign here"
                            ):
                                nc.vector.reciprocal(
                                    rc[64:65, :], ctx_ps[h][64:65, :]
                                )
                            bc = bcp.tile([64, 512], F32, tag="bc")
                            nc.tensor.matmul(
                                bc[:], _r(ones_sb[64:65, :]), rc[64:65, :],
                                start=True, stop=True,
                            )
                            rcb = nrm.tile([64, 512], F32, tag="rcb")
                            nc.vector.tensor_copy(rcb[:], bc[:])
                            nc.vector.tensor_tensor(
                                ctxT_sb[0:64, h, qg * 512:(qg + 1) * 512],
                                ctx_ps[h][0:64, :],
                                rcb[:],
                                mybir.AluOpType.mult,
                            )
                        pending = qg
                    emit_outproj(pending)

    _split_multi_waits(nc)
    return nc


_CACHED_NC = None


def _get_nc():
    global _CACHED_NC
    if _CACHED_NC is None:
        _CACHED_NC = _build_program()
    return _CACHED_NC


def _numpy_reference(q, k, v, wq, bq, wk, bk, wv, bv, wo, bo, mask):
    """Fallback for masks the fast path does not handle (non-causal)."""
    out = np.empty((B, S, D), np.float32)
    scale = 1.0 / np.sqrt(DK)
    for b in range(B):
        Q = (q[b] @ wq.T + bq).reshape(S, H, DK).transpose(1, 0, 2)
        K = (k[b] @ wk.T + bk).reshape(S, H, DK).transpose(1, 0, 2)
        V = (v[b] @ wv.T + bv).reshape(S, H, DK).transpose(1, 0, 2)
        ctx = np.empty((H, S, DK), np.float32)
        for h in range(H):
            s = (Q[h] @ K[h].T) * scale
            s = np.where(mask, s, -1e9)
            s -= s.max(axis=-1, keepdims=True)
            e = np.exp(s)
            p = e / e.sum(axis=-1, keepdims=True)
            ctx[h] = p @ V[h]
        out[b] = ctx.transpose(1, 0, 2).reshape(S, D) @ wo.T + bo
    return out


def _prepare_in_maps(q, k, v, wq, bq, wk, bk, wv, bv, wo):
    # causal 0/1 diagonal-block masks: maskc[k, j*512+q] = (128j + k) <= q
    kk = np.arange(P)[:, None]
    qq = np.arange(512)[None, :]
    maskc = np.zeros((P, 4, 512), np.float32)
    for j in range(4):
        maskc[:, j, :] = (P * j + kk) <= qq
    maskc = np.ascontiguousarray(maskc.reshape(P, 4 * 512)).astype(BF)

    wqT = np.ascontiguousarray(wq.T).astype(BF)  # [d_in, e_out]
    wkT = np.ascontiguousarray(wk.T).astype(BF)
    wvT = np.ascontiguousarray(wv.T).astype(BF)
    woT = np.ascontiguousarray(wo.T)             # [e_in, d_out]

    qTb = [np.ascontiguousarray(q[b].T).astype(BF) for b in range(B)]
    kTb = [np.ascontiguousarray(k[b].T).astype(BF) for b in range(B)]
    vTb = [np.ascontiguousarray(v[b].T).astype(BF) for b in range(B)]

    def pack_bias(bvec, scale):
        t = np.zeros((P, 2), np.float32)
        t[:, 0] = bvec[:P] * scale
        t[:64, 1] = bvec[P:E] * scale
        return t

    in_maps = []
    for c in range(NCORES):
        b = c // 4
        e0 = 3 * (c % 4) * DK
        wvp = np.zeros((D, 256), BF)
        wvp[:, :E] = wvT[:, e0:e0 + E]
        bvp = np.zeros((P, 256), np.float32)
        bvp[:, :E] = bv[e0:e0 + E][None, :]
        in_maps.append({
            "qT": qTb[b],
            "kT": kTb[b],
            "vT": vTb[b],
            "wqT": np.ascontiguousarray(wqT[:, e0:e0 + E]),
            "wkT": np.ascontiguousarray(wkT[:, e0:e0 + E]),
            "wvT": wvp,
            "woT": np.ascontiguousarray(woT[e0:e0 + E, :]).astype(BF),
            "bq": pack_bias(bq[e0:e0 + E], 0.125),
            "bk": pack_bias(bk[e0:e0 + E], 1.0),
            "bv": bvp,
            "maskc": maskc,
        })
    return in_maps


def kernel(q, k, v, wq, bq, wk, bk, wv, bv, wo, bo, mask, **_unused):
    q = np.asarray(q, np.float32)
    k = np.asarray(k, np.float32)
    v = np.asarray(v, np.float32)
    wq = np.asarray(wq, np.float32)
    wk = np.asarray(wk, np.float32)
    wv = np.asarray(wv, np.float32)
    wo = np.asarray(wo, np.float32)
    bq = np.asarray(bq, np.float32)
    bk = np.asarray(bk, np.float32)
    bv = np.asarray(bv, np.float32)
    bo = np.asarray(bo, np.float32)
    mask = np.asarray(mask)

    tril = np.tril(np.ones((S, S), bool))
    if mask.shape != (S, S) or not np.array_equal(mask.astype(bool), tril):
        return _numpy_reference(q, k, v, wq, bq, wk, bk, wv, bv, wo, bo, mask)

    in_maps = _prepare_in_maps(q, k, v, wq, bq, wk, bk, wv, bv, wo)
    nc = _get_nc()
    res = run_bass_kernel_spmd(nc, in_maps, core_ids=list(range(NCORES)))

    out = np.empty((B, S, D), np.float32)
    for b in range(B):
        acc = res.results[4 * b]["out_p"].astype(np.float32)
        for c in range(4 * b + 1, 4 * b + 4):
            acc = acc + res.results[c]["out_p"].astype(np.float32)
        out[b] = acc + bo[None, :]
    return out

